# revision 9
# baseline (speedup 1.0000x reference)
"""Trainium2 Bass kernel for nn_CubeMoveHead.

Contract: kernel(**inputs) takes the FULL unsharded inputs (as produced by
setup_inputs) and returns the FULL [512, 1536] float32 output.

Strategy (data-parallel over graphs, 64 graphs per core on 8 cores):
  Only the first 64 cube nodes of each graph ever reach the output, so the
  host computes those node indices (pure index math on cube_mask/batch),
  gathers just the needed node_features rows (4096 per core), transposes
  them to the matmul-friendly [D, nodes] layout, and ships them to each
  core's HBM in bf16. Nodes are laid out slot-major (node j on a core is
  cube slot c = j // 64 of graph g = j % 64), so the per-graph global
  feature column pattern tiles periodically: one [128, 512] bf16 plane
  (gf replicated 8x) serves every 512-node tile.

  Precision: single bf16 everywhere (x, W1a, gf, W1b, h, W2) with f32 PSUM
  accumulation measures rel err ~4e-3 vs the f32 reference on the fixed
  harness inputs -- 5x under the 2e-2 gate (numpy-verified).

  On-device per core, for each of 8 tiles of 512 node slots:
    ps1  = W1a.T @ x_t + W1b.T @ gf_rep        (2 bf16 matmuls, f32 PSUM)
    h    = relu(ps1) -> bf16                   (ACT for most tiles, DVE for
                                                some, to balance engines)
    ps2[:, 24g:24g+24] = h_grp.T @ W2          (4 tiny N=24 matmuls; h slice
                                                is the stationary operand ->
                                                FWL-fast weight loads)
    out  = min(ps2, cap)                       (cap = +BIG where slot valid
                                                & move allowed, else NEG)
  min-cap masking yields exactly NEG on masked positions; it folds b2, which
  is identically zero in the reference (asserted host-side).
"""

import sys

if "/opt/trn_rl_repo" not in sys.path:
    sys.path.insert(0, "/opt/trn_rl_repo")

import ml_dtypes
import numpy as np

import concourse.bass as bass
import concourse.mybir as mybir
from concourse.tile import TileContext
from concourse.bass_utils import run_bass_kernel_spmd

N = 500000
B = 512
D = 128
G = 128
MC = 64
M = 24
H = 128
NEG = -1.0e9
BIG = 3.0e38
NCORES = 8
GPC = B // NCORES          # graphs per core (64)
S = GPC * MC               # node slots per core (4096)
NT = S // 512              # 512-slot tiles per core (8)
PEN_W = (S // 128) * M     # cap/output plane width (768)

# consts plane layout (bf16, [128, CW]): W1a | W1b | W2 | gf_rep
CW = 128 + 128 + M + 512
OFF_WX = 0
OFF_WG = 128
OFF_W2 = 256
OFF_GF = 256 + M

# tiles whose relu runs on DVE instead of ACT (engine balancing)
DVE_RELU_TILES = (2, 5, 7)


def _legalize_single_wait(nc):
    """The walrus build here accepts at most ONE sync wait per instruction;
    Tile's scheduler happily emits several. Hoist extra waits onto same-engine
    nops inserted immediately before the offending instruction (same engine
    executes in order, so the happens-before is preserved exactly)."""
    for f in nc.m.functions:
        for bb in f.blocks:
            insts = bb.instructions
            if not any(
                i.sync_info and i.sync_info.on_wait and len(i.sync_info.on_wait) > 1
                for i in insts
            ):
                continue
            out = []
            for inst in insts:
                si = inst.sync_info
                waits = list(si.on_wait) if si and si.on_wait else []
                if len(waits) > 1:
                    for w in waits[:-1]:
                        nop = mybir.InstNoOp(
                            name=nc.get_next_instruction_name(), ins=[], outs=[]
                        )
                        nop.engine = inst.engine
                        nop.sync_info = mybir.SyncInfo(on_wait=[w], on_update=[])
                        nop.bass_nofuse = True
                        nc.register_instruction(nop)
                        out.append(nop)
                    si.on_wait = [waits[-1]]
                out.append(inst)
            bb.instructions[:] = out


def _build_program():
    f32 = mybir.dt.float32
    bf16 = mybir.dt.bfloat16
    nc = bass.Bass()
    # xz packs consts (CW cols) then the 8 x tiles (4096 cols)
    xz_d = nc.declare_dram_parameter("xz", [128, CW + S], bf16, isOutput=False)
    cap_d = nc.declare_dram_parameter("cap", [128, PEN_W], f32, isOutput=False)
    o_d = nc.declare_dram_parameter("o", [128, PEN_W], f32, isOutput=True)

    relu = mybir.ActivationFunctionType.Relu
    CA = CW + 512             # chunk A: consts + x tile 0
    CB = CW + 4 * 512         # chunk B end: x tiles 1-3

    with TileContext(nc) as tc:
        with (
            tc.tile_pool(name="consts", bufs=1) as cpool,
            tc.tile_pool(name="h", bufs=3) as hpool,
            tc.tile_pool(name="ps1", bufs=2, space="PSUM") as ps1pool,
            tc.tile_pool(name="pswarm", bufs=1, space="PSUM") as pswpool,
            tc.tile_pool(name="ps2", bufs=2, space="PSUM") as ps2pool,
            tc.tile_pool(name="o", bufs=1) as opool,
            tc.tile_pool(name="scratch", bufs=1) as spool,
        ):
            # Input DMAs first: chunk A (consts + x tile 0, small so the first
            # matmul can start early) then B (tiles 1-3) on the sync HWDGE
            # ring; C (tiles 4-7) on the scalar HWDGE ring; cap on the gpsimd
            # SWDGE ring. The three rings drain concurrently.
            xz_sb = cpool.tile([128, CW + S], bf16)
            nc.sync.dma_start(out=xz_sb[:, 0:CA], in_=xz_d[:, 0:CA])
            nc.sync.dma_start(out=xz_sb[:, CA:CB], in_=xz_d[:, CA:CB])
            nc.scalar.dma_start(out=xz_sb[:, CB:], in_=xz_d[:, CB:])
            cap_sb = cpool.tile([128, PEN_W], f32)
            nc.gpsimd.dma_start(out=cap_sb[:], in_=cap_d[:])

            # Warmups, fed by an on-chip memset (no DMA dependency): a [128,1]
            # relu so ACT's PWP table loads during the DMA wait, and a stream
            # of small dummy matmuls that keep the PE busy through the HAM
            # activity window (clock ramps 0.6 -> 2.4 GHz) without clogging
            # the PE queue ahead of the real matmuls.
            warm = spool.tile([128, 512], f32)
            nc.gpsimd.memset(warm[:], 0.0)
            nc.scalar.activation(warm[:, 0:1], warm[:, 0:1], relu)
            pswarm = pswpool.tile([128, 512], f32)
            for _ in range(20):
                nc.tensor.matmul(
                    pswarm[0:32, 0:64], warm[:, 0:32], warm[:, 0:64],
                    start=True, stop=True,
                )

            wx_sb = xz_sb[:, OFF_WX:OFF_WX + 128]
            wg_sb = xz_sb[:, OFF_WG:OFF_WG + 128]
            w2_sb = xz_sb[:, OFF_W2:OFF_W2 + M]
            gf_sb = xz_sb[:, OFF_GF:OFF_GF + 512]

            o_sb = opool.tile([128, PEN_W], f32)

            for p in range(4):
                xh = [
                    xz_sb[:, CW + (2 * p) * 512:CW + (2 * p + 1) * 512],
                    xz_sb[:, CW + (2 * p + 1) * 512:CW + (2 * p + 2) * 512],
                ]
                ps1 = [
                    ps1pool.tile([128, 512], f32, name=f"ps1{i}")
                    for i in range(2)
                ]
                # pair the mm1 matmuls so each weight set streams two tiles
                nc.tensor.matmul(ps1[0][:], wx_sb, xh[0], start=True, stop=False)
                nc.tensor.matmul(ps1[1][:], wx_sb, xh[1], start=True, stop=False)
                nc.tensor.matmul(ps1[0][:], wg_sb, gf_sb, start=False, stop=True)
                nc.tensor.matmul(ps1[1][:], wg_sb, gf_sb, start=False, stop=True)
                for i in range(2):
                    t = 2 * p + i
                    h = hpool.tile([128, 512], bf16)
                    if t in DVE_RELU_TILES:
                        nc.vector.tensor_scalar_max(h[:], ps1[i][:], 0.0)
                    else:
                        nc.scalar.activation(h[:], ps1[i][:], relu)
                    ps2 = ps2pool.tile([128, 4 * M], f32)
                    for g in range(4):
                        nc.tensor.matmul(
                            ps2[:, g * M:(g + 1) * M],
                            h[:, g * 128:(g + 1) * 128],
                            w2_sb,
                            start=True,
                            stop=True,
                        )
                    # out = min(ps2, cap): exact NEG on masked slots (b2 == 0).
                    # NOTE: scalar_tensor_tensor hangs the HW here; tensor_tensor
                    # with op=min is the verified-working form.
                    nc.vector.tensor_tensor(
                        o_sb[:, t * 4 * M:(t + 1) * 4 * M],
                        ps2[:],
                        cap_sb[:, t * 4 * M:(t + 1) * 4 * M],
                        op=mybir.AluOpType.min,
                    )
                    # drain finished output quarters early on the (idle by
                    # now) sync HWDGE ring so the final DMA is only 96 KB
                    if t in (1, 3, 5, 7):
                        q0 = (t - 1) * 4 * M
                        q1 = (t + 1) * 4 * M
                        nc.sync.dma_start(
                            out=o_d[:, q0:q1], in_=o_sb[:, q0:q1]
                        )
    _legalize_single_wait(nc)
    return nc


_NC_CACHE = None


def _get_program():
    global _NC_CACHE
    if _NC_CACHE is None:
        _NC_CACHE = _build_program()
    return _NC_CACHE


def _prepare_inputs(node_features, global_features, W1, b1, W2, b2, cube_mask,
                    batch, move_mask):
    """Host-side shard prep. Returns per-core input dicts."""
    node_features = np.asarray(node_features, dtype=np.float32)
    global_features = np.asarray(global_features, dtype=np.float32)
    W1 = np.asarray(W1, dtype=np.float32)
    b1 = np.asarray(b1, dtype=np.float32)
    W2 = np.asarray(W2, dtype=np.float32)
    b2 = np.asarray(b2, dtype=np.float32)
    cube_mask = np.asarray(cube_mask).astype(bool)
    batch = np.asarray(batch).astype(np.int64)
    move_mask = np.asarray(move_mask).astype(bool)
    assert np.all(b2 == 0.0), "kernel bakes b2==0 into the min-cap masking"
    assert np.all(b1 == 0.0), "kernel folds b1==0 into the mm1 accumulation"

    # First-64 cube nodes per graph (matches the reference's cube_idx math).
    idx = np.flatnonzero(cube_mask)                     # cube nodes, node order
    cb = batch[idx]                                     # their graph (sorted)
    counts = np.bincount(cb, minlength=B)
    starts = np.concatenate([[0], np.cumsum(counts)[:-1]])
    pos = np.arange(idx.shape[0], dtype=np.int64) - starts[cb]
    sel = pos < MC
    vidx, vb, vpos = idx[sel], cb[sel], pos[sel]

    gather_idx = np.zeros((B, MC), dtype=np.int64)
    valid = np.zeros((B, MC), dtype=bool)
    gather_idx[vb, vpos] = vidx
    valid[vb, vpos] = True

    wx = W1[:D].astype(ml_dtypes.bfloat16)              # [D, H]
    wg = W1[D:].astype(ml_dtypes.bfloat16)              # [G, H]
    w2 = W2.astype(ml_dtypes.bfloat16)                  # [H, M]

    in_maps = []
    for k in range(NCORES):
        gb = slice(k * GPC, (k + 1) * GPC)
        gi = gather_idx[gb]                             # [GPC, MC]
        # slot-major: node j = c*GPC + g  ->  (cube slot c, graph g)
        order = gi.T.reshape(-1)                        # [S]
        x_t = node_features[order].T.astype(ml_dtypes.bfloat16)   # [D, S]
        gf_rep = np.tile(
            global_features[gb].T.astype(ml_dtypes.bfloat16), (1, 512 // GPC)
        )                                               # [128, 512]
        cst = np.concatenate([wx, wg, w2, gf_rep], axis=1)        # [128, CW]
        ok = valid[gb].T.reshape(-1)[:, None] & \
            move_mask[gb].transpose(1, 0, 2).reshape(S, M)        # [S, M]
        cap = np.where(ok, np.float32(BIG), np.float32(NEG)).astype(np.float32)
        cap_dev = np.ascontiguousarray(
            cap.reshape(S // 128, 128, M).transpose(1, 0, 2).reshape(128, PEN_W)
        )
        in_maps.append({
            "xz": np.ascontiguousarray(np.concatenate([cst, x_t], axis=1)),
            "cap": cap_dev,
        })
    return in_maps


def _decode_outputs(results):
    logits = np.empty((B, MC, M), dtype=np.float32)
    for k in range(NCORES):
        o = results[k]["o"]                              # [128, PEN_W]
        scores = o.reshape(128, S // 128, M).transpose(1, 0, 2).reshape(S, M)
        # slot-major: row j = c*GPC + g
        logits[k * GPC:(k + 1) * GPC] = scores.reshape(MC, GPC, M).transpose(1, 0, 2)
    return logits.reshape(B, MC * M)


def kernel(**inputs) -> np.ndarray:
    in_maps = _prepare_inputs(**inputs)
    nc = _get_program()
    res = run_bass_kernel_spmd(nc, in_maps, list(range(NCORES)))
    return _decode_outputs(res.results)
